# revision 1
# baseline (speedup 1.0000x reference)
"""Trainium2 Bass kernel for nn_EncodingNet (FastGTN-style GNN).

Self-contained: the host shards/packs inputs (index bucketing + repacking
only -- no value arithmetic), builds + runs an 8-core SPMD Bass kernel via
PJRT (axon), and gathers the full output.

Algorithmic structure (operator form -- never materializes mats1 @ mats0):
  E_t = densify(edge_index[t], edge_value[t])        [2048, 2048] per type
  mats_l[c] = sum_t softmax(conv_w[l])[c,t] * E_t    (materialized per core
              as row-shards in SBUF, fp32, built from int16 scattered E)
  6 sequential row-parallel GEMM passes over mats cover GT layer 0, GT
  layer 1, GCN1 (mats0, mats1), GCN2 (mats0, mats1); an AllGather after
  each pass rebuilds the full-height RHS for the next.

Sharding: nodes row-sharded over 8 cores (256 rows/core). Edge values are
scatter-packed via gpsimd local_scatter as int16 fixed point (duplicates
are summed on-device in fp32 before quantization); the dequant scale is
folded into the softmax(conv_w) coefficients.
"""

import os
import sys
import types

import numpy as np

# ---------------------------------------------------------------------------
# Environment workaround (inline: kernel.py must be self-contained).
# ---------------------------------------------------------------------------
if "antenv.axon_hooks" not in sys.modules:
    _m = types.ModuleType("antenv.axon_hooks")
    _m.get_axon_ntff_profile_hook = lambda: None
    sys.modules["antenv.axon_hooks"] = _m

import concourse.bass as bass
import concourse.bacc as bacc
import concourse.tile as tile
from concourse import mybir

# ---------------------------------------------------------------------------
# Problem constants (hardcoded per the task contract).
# ---------------------------------------------------------------------------
N = 2048          # nodes
C = 2             # channels
T = 3             # edge types
L = 2             # GT layers
E = 65536         # edges per type
W_IN = 256
W_OUT = 64
NCLS = 16
NTGT = 512
BETA = 0.5

NCORES = 8
NS = N // NCORES  # 256 rows per core
P = 128
KC = 16           # node chunks: node j = p*16 + kc
NEL = 1024        # local_scatter num_elems per call
NCH = (T * KC * NS) // NEL  # 12 scatter chunks
EFREE = T * KC * NS         # 12288 free elems of the E^T SBUF tile
MFREE = KC * NS             # 4096 free elems of one mats tile

f32 = mybir.dt.float32
i16 = mybir.dt.int16
i32 = mybir.dt.int32
AF = mybir.ActivationFunctionType
OP = mybir.AluOpType

# misc pack offsets (columns in the [128, MISC_W] fp32 misc tensor)
_MO_CONV = 0          # [12]
_MO_B1 = 12           # [16]
_MO_B2 = 28           # [64]
_MO_LB = 92           # [16]
_MO_ID = 108          # [128]
_MO_GW1 = 236         # [16]  (partitions 0..63)
_MO_GW2 = 252         # [64]  (partitions 0..15)
_MO_LW = 316          # [16]  (partitions 0..63)
_MO_TIDX = 332        # [4]   (int32 bits)
MISC_W = 336


# ---------------------------------------------------------------------------
# Host-side packing.
# ---------------------------------------------------------------------------
def _prep_inputs(X, edge_value, conv_w, Ws, gcn_w1, gcn_b1, gcn_w2, gcn_b2,
                 lin_w, lin_b, edge_index, target_x):
    X = np.asarray(X, np.float32)
    edge_value = np.asarray(edge_value, np.float32)
    conv_w = np.asarray(conv_w, np.float32)
    Ws = np.asarray(Ws, np.float32)
    gcn_w1 = np.asarray(gcn_w1, np.float32)
    gcn_b1 = np.asarray(gcn_b1, np.float32)
    gcn_w2 = np.asarray(gcn_w2, np.float32)
    gcn_b2 = np.asarray(gcn_b2, np.float32)
    lin_w = np.asarray(lin_w, np.float32)
    lin_b = np.asarray(lin_b, np.float32)
    ei = np.asarray(edge_index, np.int64)
    tx = np.asarray(target_x, np.int64)

    # xT_perm[:, kc*128 + p] = X.T[:, p*16 + kc]  (node j = p*16 + kc)
    kk, pp = np.meshgrid(np.arange(KC), np.arange(P), indexing="ij")
    pos_node = (pp * KC + kk).reshape(-1)
    xT_perm = np.ascontiguousarray(X[pos_node].T)          # [256, 2048]
    ws_cat = np.concatenate([Ws[0], Ws[1]], axis=1)        # [256, 128]

    def fold(a):  # [256, F] -> [128, 2*F]  (feat = kf*128 + p)
        fdim = a.shape[1]
        return np.ascontiguousarray(
            a.reshape(2, P, fdim).transpose(1, 0, 2).reshape(P, 2 * fdim))

    misc = np.zeros((P, MISC_W), np.float32)
    misc[:, _MO_CONV:_MO_CONV + 12] = conv_w.reshape(1, -1)
    misc[:, _MO_B1:_MO_B1 + 16] = gcn_b1.reshape(1, -1)
    misc[:, _MO_B2:_MO_B2 + 64] = gcn_b2.reshape(1, -1)
    misc[:, _MO_LB:_MO_LB + 16] = lin_b.reshape(1, -1)
    misc[:, _MO_ID:_MO_ID + 128] = np.eye(P, dtype=np.float32)
    misc[:64, _MO_GW1:_MO_GW1 + 16] = gcn_w1
    misc[:16, _MO_GW2:_MO_GW2 + 64] = gcn_w2
    misc[:64, _MO_LW:_MO_LW + 16] = lin_w
    tidx = np.ascontiguousarray(
        tx.reshape(NTGT // P, P).T.astype(np.int32))       # [128, 4]
    misc[:, _MO_TIDX:_MO_TIDX + 4] = tidx.view(np.float32)

    # ---- edge bucketing per core (vectorized, index-only) -----------------
    t_id = np.repeat(np.arange(T, dtype=np.int64), E)
    r_all = ei[:, 0, :].reshape(-1)
    c_all = ei[:, 1, :].reshape(-1)
    v_all = edge_value.reshape(-1)
    rank = r_all >> 8
    r_loc = r_all & 255
    p_of = c_all >> 4
    kc_of = c_all & 15
    free = t_id * MFREE + kc_of * NS + r_loc
    ch_of = free // NEL
    pos_of = free % NEL
    bucket = (rank * P + p_of) * NCH + ch_of
    cell = bucket * NEL + pos_of

    order = np.argsort(cell, kind="stable")
    cell_s = cell[order]
    v_s = v_all[order]
    ucell, first_idx, counts = np.unique(cell_s, return_index=True,
                                         return_counts=True)
    occ = np.arange(len(cell_s)) - np.repeat(first_idx, counts)
    M = int(counts.max())
    ubucket = ucell // NEL
    upos = ucell % NEL
    # order unique cells within each bucket with duplicated cells FIRST so
    # the device only runs the dup-sum adds over a tiny slot window
    order2 = np.lexsort((np.arange(len(ucell)), counts == 1, ubucket))
    inv2 = np.empty_like(order2)
    inv2[order2] = np.arange(len(order2))
    ub_sorted = ubucket[order2]
    ub_uniq, ub_fidx, ub_counts = np.unique(ub_sorted, return_index=True,
                                            return_counts=True)
    slot_sorted = np.arange(len(ucell)) - np.repeat(ub_fidx, ub_counts)
    slot = slot_sorted[inv2]
    ndup_per_bucket = np.zeros(len(ub_uniq), np.int64)
    isdup_sorted = (counts[order2] >= 2)
    np.add.at(ndup_per_bucket,
              np.searchsorted(ub_uniq, ub_sorted), isdup_sorted)
    DUPW = int(ndup_per_bucket.max()) if M > 1 else 0
    max_cnt = int(ub_counts.max())
    NI = max_cnt + (max_cnt & 1)
    scale = float(2.0 ** np.floor(np.log2(32767.0 / M)))

    scat_idx = np.full((NCORES, P, NCH * NI), -1, np.int16)
    scat_vals = np.zeros((NCORES, M, P, NCH * NI), np.float32)
    uk = ubucket // (P * NCH)
    up = (ubucket // NCH) % P
    uch = ubucket % NCH
    scat_idx[uk, up, uch * NI + slot] = upos.astype(np.int16)
    cell_row = np.searchsorted(ucell, cell_s)
    scat_vals[uk[cell_row], occ, up[cell_row],
              uch[cell_row] * NI + slot[cell_row]] = v_s

    big0_shared = np.concatenate([fold(xT_perm), fold(ws_cat)], axis=1)
    in_maps = []
    for k in range(NCORES):
        xmy = fold(np.ascontiguousarray(X[k * NS:(k + 1) * NS].T))
        m = {
            "big0": np.ascontiguousarray(
                np.concatenate([big0_shared, xmy], axis=1)),
            "misc": misc,
            "sidx": scat_idx[k],
            "svals": np.ascontiguousarray(
                scat_vals[k].transpose(1, 0, 2).reshape(P, -1)),
        }
        in_maps.append(m)
    return in_maps, NI, M, scale, DUPW


# ---------------------------------------------------------------------------
# Device kernel.
# ---------------------------------------------------------------------------
class _StageStop(Exception):
    pass


def build_kernel(NI, M, scale, DUPW=0, reps=1, stop_after=None):
    nc = bacc.Bacc("TRN2", target_bir_lowering=False, debug=False,
                   num_devices=NCORES)
    F = NCH * NI
    # big0 layout: [xT fold (4096) | ws fold (256) | xmy fold (512)]
    XT_OFF = 0
    WS_OFF = 2 * N
    XMY_OFF = 2 * N + 2 * C * W_OUT
    BIG0_W = XMY_OFF + 2 * NS

    big0_d = nc.dram_tensor("big0", [P, BIG0_W], f32, kind="ExternalInput")
    misc_d = nc.dram_tensor("misc", [P, MISC_W], f32, kind="ExternalInput")
    sidx_d = nc.dram_tensor("sidx", [P, F], i16, kind="ExternalInput")
    svals_d = nc.dram_tensor("svals", [P, M * F], f32, kind="ExternalInput")
    y_d = nc.dram_tensor("y", [NTGT, NCLS], f32, kind="ExternalOutput")

    ccds = []
    for r in range(reps):
        ccd = {}
        for name, d in [("A", 130), ("C", 16), ("D", C * 16),
                        ("E", W_OUT), ("F", C * W_OUT), ("H", W_OUT)]:
            ccd[name] = (
                nc.dram_tensor(f"cci_{name}{r}", [NS, d], f32),
                nc.dram_tensor(f"cco_{name}{r}", [N, d], f32,
                               addr_space="Shared"),
                d,
            )
        ccds.append(ccd)
    rg = [list(range(NCORES))]

    with tile.TileContext(nc) as tc:
        import contextlib
        ctx = contextlib.ExitStack()
        with ctx:
            pool = ctx.enter_context(tc.tile_pool(name="main", bufs=1))
            ppool = ctx.enter_context(
                tc.tile_pool(name="pass_psum", bufs=4, space="PSUM"))
            apool = ctx.enter_context(
                tc.tile_pool(name="aux_psum", bufs=3, space="PSUM"))

            # ---------------- consolidated input loads ----------------
            # scatter-path inputs first: they gate the serial E-build chain
            misc = pool.tile([P, MISC_W], f32, tag="misc")
            nc.sync.dma_start(misc[:], misc_d[:])
            sidx_sb = pool.tile([P, F], i16, tag="sidx")
            nc.sync.dma_start(sidx_sb[:], sidx_d[:])
            svals_sb = pool.tile([P, M * F], f32, tag="svals")
            nc.sync.dma_start(svals_sb[:], svals_d[:])
            big0 = pool.tile([P, BIG0_W], f32, tag="big0")
            nc.sync.dma_start(big0[:], big0_d[:])

            ident = misc[:, _MO_ID:_MO_ID + 128]
            b1_ap = misc[:, _MO_B1:_MO_B1 + 16]
            b2_ap = misc[:, _MO_B2:_MO_B2 + 64]
            lb_ap = misc[:, _MO_LB:_MO_LB + 16]
            gw1_ap = misc[0:64, _MO_GW1:_MO_GW1 + 16]
            gw2_ap = misc[0:16, _MO_GW2:_MO_GW2 + 64]
            lw_ap = misc[0:64, _MO_LW:_MO_LW + 16]
            tidx_ap = misc[:, _MO_TIDX:_MO_TIDX + 4].bitcast(i32)

            prev_y = None
            stage_state = {}

            def _stage(name, tile_ref):
                stage_state["last"] = tile_ref
                if stop_after == name:
                    raise _StageStop()

            for rep in range(reps):
                try:
                    # ---------- filt = softmax(conv_w) / scale ----------
                    ex = pool.tile([P, L * C * T], f32, tag="ex")
                    nc.scalar.activation(ex[:],
                                         misc[:, _MO_CONV:_MO_CONV + 12],
                                         AF.Exp)
                    sums = pool.tile([P, L * C], f32, tag="sums")
                    nc.vector.tensor_reduce(
                        sums[:], ex[:].rearrange("p (g t) -> p g t", t=T),
                        axis=mybir.AxisListType.X, op=OP.add)
                    rec = pool.tile([P, L * C], f32, tag="rec")
                    nc.vector.reciprocal(rec[:], sums[:])
                    filt = pool.tile([P, L * C * T], f32, tag="filt")
                    for g in range(L * C):
                        nc.vector.tensor_scalar_mul(
                            filt[:, g * T:(g + 1) * T],
                            ex[:, g * T:(g + 1) * T], rec[:, g:g + 1])
                    filt_s = pool.tile([P, L * C * T], f32, tag="filt_s")
                    nc.scalar.activation(filt_s[:], filt[:], AF.Copy, bias=0.0,
                                         scale=1.0 / scale)

                    def fs(l, c, t):
                        q = (l * C + c) * T + t
                        return filt_s[:, q:q + 1]

                    # ---------- E build: quantize + scatter ----------
                    # dup cells sit in slots [0, DUPW) of each chunk: sum the
                    # m>=1 value planes into plane 0 over that window only.
                    svv = svals_sb[:].rearrange("p (m c s) -> p m c s",
                                                m=M, c=NCH)
                    vsum = svv[:, 0, :, :].rearrange("p c s -> p (c s)")
                    if M > 1 and DUPW > 0 and rep == 0:
                        d0 = svv[:, 0, :, 0:DUPW]
                        for m in range(1, M):
                            nc.vector.tensor_add(d0, d0,
                                                 svv[:, m, :, 0:DUPW])
                    if prev_y is not None:
                        jz = pool.tile([P, 1], f32, tag="jz")
                        nc.vector.tensor_scalar_mul(jz[:], prev_y, 0.0)
                        nc.vector.tensor_scalar_add(vsum[:, 0:1],
                                                    vsum[:, 0:1], jz[:, :])
                    vq_f = pool.tile([P, F], f32, tag="vq_f")
                    nc.scalar.activation(vq_f[:], vsum, AF.Copy, bias=0.0,
                                         scale=scale)
                    vq = pool.tile([P, F], i16, tag="vq")
                    nc.vector.tensor_copy(vq[:], vq_f[:])

                    eqh = [pool.tile([P, T * 2048], i16, tag=f"eq{hh}",
                                     name=f"eq{hh}") for hh in range(2)]
                    # scatter in q-major order so mats halves start early
                    for q in range(NCH // T):
                        for t in range(T):
                            ch = t * (NCH // T) + q
                            dst = eqh[q // 2][:, t * 2048 + (q % 2) * NEL:
                                              t * 2048 + (q % 2) * NEL + NEL]
                            nc.gpsimd.local_scatter(
                                out_ap=dst,
                                data_ap=vq[:, ch * NI:(ch + 1) * NI],
                                idxs_ap=sidx_sb[:, ch * NI:(ch + 1) * NI],
                                channels=P, num_elems=NEL, num_idxs=NI)

                    # ------- mats_l[c] = sum_t filt_s[l,c,t] * E_t -------
                    # mats0 now (gates pass A); mats1 is emitted after pass
                    # A so it overlaps pass A + the first AllGather. Each
                    # (l, c) is a pair of half tiles [P, 2048] (kc 0-7 /
                    # 8-15) so the build pipelines with the scatters.
                    mats = [[[pool.tile([P, MFREE // 2], f32,
                                        tag=f"mats{l}{c}{hh}",
                                        name=f"mats{l}{c}{hh}")
                              for hh in range(2)]
                             for c in range(C)] for l in range(L)]

                    def build_mats(l):
                        for hh in range(2):
                            for c in range(C):
                                dst = mats[l][c][hh][:]
                                nc.vector.tensor_scalar_mul(
                                    dst, eqh[hh][:, 0:2048], fs(l, c, 0))
                                for t in range(1, T):
                                    nc.vector.scalar_tensor_tensor(
                                        out=dst,
                                        in0=eqh[hh][:, t * 2048:
                                                    (t + 1) * 2048],
                                        scalar=fs(l, c, t), in1=dst,
                                        op0=OP.mult, op1=OP.add)

                    build_mats(0)
                    _stage("ebuild", mats[0][0][0][:, 0:1])

                    def mchunk(l, c, kc, mb):
                        # kc 0-7 in half 0 (q 0,1), kc 8-15 in half 1.
                        # within half: free = (kc % 8) * NS + r
                        o = (kc % 8) * NS + mb * P
                        return mats[l][c][kc // 8][:, o:o + P]

                    # ---------- X_ = X @ Ws -> rhs_a [X0|1|X1|1] ----------
                    rhs_a = pool.tile([P, KC, 130], f32, tag="rhs_a")
                    nc.vector.memset(rhs_a[:], 1.0)
                    for kc in range(KC):
                        ps = apool.tile([P, C * W_OUT], f32, space="PSUM",
                                        tag="aux")
                        for a in range(2):
                            nc.tensor.matmul(
                                ps[:],
                                big0[:, XT_OFF + a * N + kc * P:
                                     XT_OFF + a * N + (kc + 1) * P],
                                big0[:, WS_OFF + a * C * W_OUT:
                                     WS_OFF + (a + 1) * C * W_OUT],
                                start=(a == 0), stop=(a == 1))
                        nc.vector.tensor_copy(
                            rhs_a[:, kc, :].rearrange(
                                "p (b q) -> p b q", q=65)[:, :, 0:64],
                            ps[:].rearrange("p (b q) -> p b q", q=64))
                    xmy_sb = pool.tile([P, 2, C * W_OUT], f32, tag="xmy")
                    for mb in range(2):
                        ps = apool.tile([P, C * W_OUT], f32, space="PSUM",
                                        tag="aux")
                        for a in range(2):
                            nc.tensor.matmul(
                                ps[:],
                                big0[:, XMY_OFF + a * NS + mb * P:
                                     XMY_OFF + a * NS + (mb + 1) * P],
                                big0[:, WS_OFF + a * C * W_OUT:
                                     WS_OFF + (a + 1) * C * W_OUT],
                                start=(a == 0), stop=(a == 1))
                        nc.vector.tensor_copy(xmy_sb[:, mb, :], ps[:])

                    def allgather(name, shard_sb):
                        cci, cco, d = ccds[rep][name]
                        cciv = cci[:].rearrange("(mb p) d -> mb p d", p=P)
                        for mb in range(2):
                            nc.sync.dma_start(cciv[mb], shard_sb[:, mb, :])
                        nc.gpsimd.collective_compute(
                            "AllGather", OP.bypass, replica_groups=rg,
                            ins=[cci[:]], outs=[cco[:]])
                        rhs = pool.tile([P, KC, d], f32, tag=f"rhs_{name}")
                        nc.sync.dma_start(
                            rhs[:],
                            cco[:].rearrange("(p k) d -> p k d", p=P))
                        return rhs

                    # ================ PASS A (GT layer 0) ================
                    shA = pool.tile([P, 2, 130], f32, tag="shA")
                    for c in range(C):
                        for mb in range(2):
                            ps = ppool.tile([P, 65], f32, space="PSUM",
                                            tag="ep")
                            for kc in range(KC):
                                nc.tensor.matmul(
                                    ps[:], mchunk(0, c, kc, mb),
                                    rhs_a[:, kc, 65 * c:65 * c + 65],
                                    start=(kc == 0), stop=(kc == KC - 1))
                            nc.vector.tensor_copy(
                                shA[:, mb, 65 * c:65 * c + 65], ps[:])
                    build_mats(1)   # overlaps pass A + AllGather A
                    rhs_b = allgather("A", shA)
                    _stage("passA", rhs_b[:, 0, 0:1])

                    # ================ PASS B (GT layer 1) ================
                    psB = [[None] * 2 for _ in range(C)]
                    for c in range(C):
                        for mb in range(2):
                            ps = ppool.tile([P, 65], f32, space="PSUM",
                                            tag="ep")
                            psB[c][mb] = ps
                            for kc in range(KC):
                                nc.tensor.matmul(
                                    ps[:], mchunk(1, c, kc, mb),
                                    rhs_b[:, kc, 65 * c:65 * c + 65],
                                    start=(kc == 0), stop=(kc == KC - 1))
                    dinv = pool.tile([P, 2, 1], f32, tag="dinv")
                    hc_sb = pool.tile([P, 2, W_OUT], f32, tag="hc")
                    hcT_sb = pool.tile([W_OUT, NS], f32, tag="hcT")
                    w1_sb = pool.tile([P, 2, 16], f32, tag="w1")
                    for mb in range(2):
                        dg = pool.tile([P, 1], f32, tag="deg", bufs=2)
                        nc.vector.tensor_scalar_add(dg[:],
                                                    psB[0][mb][:, 64:65], 1.0)
                        nc.vector.tensor_add(dg[:], dg[:],
                                             psB[1][mb][:, 64:65])
                        sq = pool.tile([P, 1], f32, tag="sq", bufs=2)
                        nc.scalar.activation(sq[:], dg[:], AF.Sqrt)
                        nc.vector.reciprocal(dinv[:, mb, :], sq[:])
                        rsum = pool.tile([P, W_OUT], f32, tag="rsum", bufs=2)
                        for c in range(C):
                            tmp = pool.tile([P, W_OUT], f32, tag="hctmp",
                                            bufs=2)
                            nc.vector.tensor_add(
                                tmp[:], xmy_sb[:, mb, 64 * c:64 * c + 64],
                                psB[c][mb][:, 0:64])
                            if c == 0:
                                nc.scalar.activation(rsum[:], tmp[:],
                                                     AF.Relu, scale=BETA)
                            else:
                                r2 = pool.tile([P, W_OUT], f32,
                                               tag="hctmp2", bufs=2)
                                nc.scalar.activation(r2[:], tmp[:], AF.Relu,
                                                     scale=BETA)
                                nc.vector.tensor_add(rsum[:], rsum[:], r2[:])
                        nc.vector.tensor_scalar_mul(hc_sb[:, mb, :], rsum[:],
                                                    0.5)
                        tp = apool.tile([P, P], f32, space="PSUM", tag="aux")
                        nc.tensor.transpose(tp[:W_OUT, :], hc_sb[:, mb, :],
                                            ident)
                        nc.vector.tensor_copy(
                            hcT_sb[:, mb * P:(mb + 1) * P], tp[:W_OUT, :])
                    for mb in range(2):
                        psz = apool.tile([P, 16], f32, space="PSUM",
                                         tag="aux")
                        nc.tensor.matmul(psz[:],
                                         hcT_sb[:, mb * P:(mb + 1) * P],
                                         gw1_ap, start=True, stop=True)
                        nc.vector.tensor_scalar_mul(w1_sb[:, mb, :], psz[:],
                                                    dinv[:, mb, :])
                    rhs_c = allgather("C", w1_sb)
                    _stage("passB", rhs_c[:, 0, 0:1])

                    # ================ PASS C (GCN1 mats0) ================
                    shC = pool.tile([P, 2, C * 16], f32, tag="shC")
                    for c in range(C):
                        for mb in range(2):
                            ps = ppool.tile([P, 16], f32, space="PSUM",
                                            tag="ep")
                            for kc in range(KC):
                                nc.tensor.matmul(
                                    ps[:], mchunk(0, c, kc, mb),
                                    rhs_c[:, kc, :],
                                    start=(kc == 0), stop=(kc == KC - 1))
                            nc.vector.tensor_copy(
                                shC[:, mb, 16 * c:16 * c + 16], ps[:])
                    rhs_d = allgather("D", shC)
                    _stage("passC", rhs_d[:, 0, 0:1])

                    # ================ PASS D (GCN1 mats1) ================
                    h_sb = pool.tile([P, 2, 16], f32, tag="h")
                    hT_sb = pool.tile([16, NS], f32, tag="hT")
                    w2_sb = pool.tile([P, 2, W_OUT], f32, tag="w2")
                    for mb in range(2):
                        ps = ppool.tile([P, 16], f32, space="PSUM", tag="ep")
                        first = True
                        for c in range(C):
                            for kc in range(KC):
                                nc.tensor.matmul(
                                    ps[:], mchunk(1, c, kc, mb),
                                    rhs_d[:, kc, 16 * c:16 * c + 16],
                                    start=first,
                                    stop=(c == C - 1 and kc == KC - 1))
                                first = False
                        aw = pool.tile([P, 16], f32, tag="aw1", bufs=2)
                        nc.vector.tensor_add(aw[:], ps[:], w1_sb[:, mb, :])
                        nc.vector.tensor_scalar_mul(aw[:], aw[:],
                                                    dinv[:, mb, :])
                        nc.vector.tensor_add(aw[:], aw[:], b1_ap)
                        nc.vector.tensor_scalar_max(h_sb[:, mb, :], aw[:],
                                                    0.0)
                        tp = apool.tile([P, P], f32, space="PSUM", tag="aux")
                        nc.tensor.transpose(tp[:16, :], h_sb[:, mb, :],
                                            ident)
                        nc.vector.tensor_copy(
                            hT_sb[:, mb * P:(mb + 1) * P], tp[:16, :])
                    for mb in range(2):
                        psz = apool.tile([P, W_OUT], f32, space="PSUM",
                                         tag="aux")
                        nc.tensor.matmul(psz[:],
                                         hT_sb[:, mb * P:(mb + 1) * P],
                                         gw2_ap, start=True, stop=True)
                        nc.vector.tensor_scalar_mul(w2_sb[:, mb, :], psz[:],
                                                    dinv[:, mb, :])
                    rhs_e = allgather("E", w2_sb)
                    _stage("passD", rhs_e[:, 0, 0:1])

                    # ================ PASS E (GCN2 mats0) ================
                    shE = pool.tile([P, 2, C * W_OUT], f32, tag="shE")
                    for c in range(C):
                        for mb in range(2):
                            ps = ppool.tile([P, W_OUT], f32, space="PSUM",
                                            tag="ep")
                            for kc in range(KC):
                                nc.tensor.matmul(
                                    ps[:], mchunk(0, c, kc, mb),
                                    rhs_e[:, kc, :],
                                    start=(kc == 0), stop=(kc == KC - 1))
                            nc.vector.tensor_copy(
                                shE[:, mb, 64 * c:64 * c + 64], ps[:])
                    rhs_f = allgather("F", shE)
                    _stage("passE", rhs_f[:, 0, 0:1])

                    # ========== PASS F (GCN2 mats1) + log_softmax ==========
                    hls_sb = pool.tile([P, 2, W_OUT], f32, tag="hls")
                    for mb in range(2):
                        ps = ppool.tile([P, W_OUT], f32, space="PSUM",
                                        tag="ep")
                        first = True
                        for c in range(C):
                            for kc in range(KC):
                                nc.tensor.matmul(
                                    ps[:], mchunk(1, c, kc, mb),
                                    rhs_f[:, kc, 64 * c:64 * c + 64],
                                    start=first,
                                    stop=(c == C - 1 and kc == KC - 1))
                                first = False
                        aw = pool.tile([P, W_OUT], f32, tag="aw2", bufs=2)
                        nc.vector.tensor_add(aw[:], ps[:], w2_sb[:, mb, :])
                        nc.vector.tensor_scalar_mul(aw[:], aw[:],
                                                    dinv[:, mb, :])
                        nc.vector.tensor_add(aw[:], aw[:], b2_ap)
                        mx = pool.tile([P, 1], f32, tag="mx", bufs=2)
                        nc.vector.tensor_reduce(mx[:], aw[:],
                                                axis=mybir.AxisListType.X,
                                                op=OP.max)
                        nmx = pool.tile([P, 1], f32, tag="nmx", bufs=2)
                        nc.vector.tensor_scalar_mul(nmx[:], mx[:], -1.0)
                        ee = pool.tile([P, W_OUT], f32, tag="ee", bufs=2)
                        nc.scalar.activation(ee[:], aw[:], AF.Exp,
                                             bias=nmx[:, :])
                        ssum = pool.tile([P, 1], f32, tag="ssum", bufs=2)
                        nc.vector.tensor_reduce(ssum[:], ee[:],
                                                axis=mybir.AxisListType.X,
                                                op=OP.add)
                        lns = pool.tile([P, 1], f32, tag="lns", bufs=2)
                        nc.scalar.activation(lns[:], ssum[:], AF.Ln)
                        tot = pool.tile([P, 1], f32, tag="tot", bufs=2)
                        nc.vector.tensor_add(tot[:], mx[:], lns[:])
                        nc.vector.tensor_scalar(out=hls_sb[:, mb, :],
                                                in0=aw[:], scalar1=tot[:, :],
                                                scalar2=None,
                                                op0=OP.subtract)

                    # -------- AG h, gather targets, linear head --------
                    cci, cco, _ = ccds[rep]["H"]
                    cciv = cci[:].rearrange("(mb p) d -> mb p d", p=P)
                    for mb in range(2):
                        nc.sync.dma_start(cciv[mb], hls_sb[:, mb, :])
                    nc.gpsimd.collective_compute(
                        "AllGather", OP.bypass, replica_groups=rg,
                        ins=[cci[:]], outs=[cco[:]])
                    hloc = nc.dram_tensor(f"hloc{rep}", [N, W_OUT], f32)
                    nc.sync.dma_start(hloc[:], cco[:])
                    gt = pool.tile([P, NTGT // P, W_OUT], f32, tag="gt")
                    for b in range(NTGT // P):
                        nc.gpsimd.indirect_dma_start(
                            out=gt[:, b, :], out_offset=None, in_=hloc[:],
                            in_offset=bass.IndirectOffsetOnAxis(
                                ap=tidx_ap[:, b:b + 1], axis=0))
                    gT_sb = pool.tile([W_OUT, NTGT], f32, tag="gT")
                    for b in range(NTGT // P):
                        tp = apool.tile([P, P], f32, space="PSUM", tag="aux")
                        nc.tensor.transpose(tp[:W_OUT, :], gt[:, b, :],
                                            ident)
                        nc.vector.tensor_copy(
                            gT_sb[:, b * P:(b + 1) * P], tp[:W_OUT, :])
                    y_sb = pool.tile([P, NTGT // P, NCLS], f32, tag="y_sb")
                    for b in range(NTGT // P):
                        psy = apool.tile([P, NCLS], f32, space="PSUM",
                                         tag="aux")
                        nc.tensor.matmul(psy[:],
                                         gT_sb[:, b * P:(b + 1) * P],
                                         lw_ap, start=True, stop=True)
                        nc.vector.tensor_add(y_sb[:, b, :], psy[:], lb_ap)
                    yv = y_d[:].rearrange("(b p) n -> b p n", p=P)
                    for b in range(NTGT // P):
                        nc.sync.dma_start(yv[b], y_sb[:, b, :])
                except _StageStop:
                    lt = stage_state["last"]
                    y_sb = pool.tile([P, NTGT // P, NCLS], f32, tag="ydummy")
                    nc.vector.memset(y_sb[:], 0.0)
                    nc.vector.tensor_scalar_mul(y_sb[:, 0, 0:1], lt, 0.0)
                    yv = y_d[:].rearrange("(b p) n -> b p n", p=P)
                    for b in range(NTGT // P):
                        nc.sync.dma_start(yv[b], y_sb[:, b, :])
                prev_y = y_sb[:, 0, 0:1]

    nc.compile()
    return nc


# ---------------------------------------------------------------------------
# Execution via PJRT (axon) with a persistent jitted callable.
# ---------------------------------------------------------------------------
class _Runner:
    def __init__(self, nc, n_cores):
        import jax
        from jax.sharding import Mesh, PartitionSpec
        from jax.experimental.shard_map import shard_map
        from concourse.bass2jax import (
            _bass_exec_p, install_neuronx_cc_hook, partition_id_tensor)

        install_neuronx_cc_hook()
        self.jax = jax
        self._nc = nc
        self.n_cores = n_cores
        partition_name = (
            nc.partition_id_tensor.name if nc.partition_id_tensor else None)
        in_names, out_names, out_avals, zero_outs = [], [], [], []
        for alloc in nc.m.functions[0].allocations:
            if not isinstance(alloc, mybir.MemoryLocationSet):
                continue
            name = alloc.memorylocations[0].name
            if alloc.kind == "ExternalInput":
                if name != partition_name:
                    in_names.append(name)
            elif alloc.kind == "ExternalOutput":
                shape = tuple(alloc.tensor_shape)
                dtype = mybir.dt.np(alloc.dtype)
                out_names.append(name)
                out_avals.append(jax.core.ShapedArray(shape, dtype))
                zero_outs.append(np.zeros(shape, dtype))
        self.n_params = len(in_names)
        self.out_names = out_names
        self.out_avals = out_avals
        self.zero_outs = zero_outs
        n_outs = len(out_avals)
        in_names = in_names + out_names
        if partition_name is not None:
            in_names.append(partition_name)
        self.in_names = in_names

        def _body(*args):
            operands = list(args)
            if partition_name is not None:
                operands.append(partition_id_tensor())
            outs = _bass_exec_p.bind(
                *operands, out_avals=tuple(out_avals),
                in_names=tuple(in_names), out_names=tuple(out_names),
                lowering_input_output_aliases=(),
                sim_require_finite=True, sim_require_nnan=True, nc=nc)
            return tuple(outs)

        devices = jax.devices()[:n_cores]
        mesh = Mesh(np.asarray(devices), ("core",))
        in_specs = (PartitionSpec("core"),) * (self.n_params + n_outs)
        out_specs = (PartitionSpec("core"),) * n_outs
        self._fn = jax.jit(
            shard_map(_body, mesh=mesh, in_specs=in_specs,
                      out_specs=out_specs, check_rep=False),
            donate_argnums=tuple(range(self.n_params,
                                       self.n_params + n_outs)),
            keep_unused=True)

    def concat_inputs(self, in_maps):
        return [
            np.concatenate([np.asarray(m[name]) for m in in_maps], axis=0)
            for name in self.in_names[: self.n_params]
        ]

    def zeros(self):
        return [
            np.zeros((self.n_cores * z.shape[0], *z.shape[1:]), z.dtype)
            for z in self.zero_outs
        ]

    def run(self, in_maps):
        outs = self._fn(*self.concat_inputs(in_maps), *self.zeros())
        return [
            {
                name: np.asarray(outs[i]).reshape(
                    self.n_cores, *self.out_avals[i].shape)[c]
                for i, name in enumerate(self.out_names)
            }
            for c in range(self.n_cores)
        ]


_CACHE = {}


def _get_runner(NI, M, scale, DUPW=0, reps=1, stop_after=None):
    key = (NI, M, scale, DUPW, reps, stop_after)
    if key not in _CACHE:
        nc = build_kernel(NI, M, scale, DUPW=DUPW, reps=reps,
                          stop_after=stop_after)
        _CACHE[key] = _Runner(nc, NCORES)
    return _CACHE[key]


def kernel(**inputs) -> np.ndarray:
    in_maps, NI, M, scale, DUPW = _prep_inputs(**inputs)
    runner = _get_runner(NI, M, scale, DUPW)
    results = runner.run(in_maps)
    return results[0]["y"]



# revision 2
# speedup vs baseline: 4.4917x; 4.4917x over previous
"""Trainium2 Bass kernel v2 for nn_EncodingNet (FastGTN-style GNN).

Structure (operator form; never materializes dense mats products):
  E_t = densify(edge_index[t], edge_value[t]) as fp16, scattered twice:
    E0: row-sharded   (core k owns rows j with (j%16)//2 == k)
    E1: column-sharded (core k owns cols c with (c%16)//2 == k)
  Each operator application  sum_c mats1[c] @ f(mats0[c] @ V)  becomes a
  LOCAL pair of GEMM passes: pass-0 contracts E0 rows (full RHS needed),
  pass-1 contracts E1 columns against pass-0's local output, yielding a
  full-height PARTIAL that a ReduceScatter sums. Per-type filt coefficients
  are folded into scaled fp16 RHS copies so E_t feeds the PE directly.

  Rounds:  A0/A1 -> RS1 -> local Hc/W1/dinv -> AG1 ->
           C0/D1 -> RS2 -> local h/W2       -> AG2 ->
           E0/F1 -> RS3 -> local log_softmax + per-core target head.
  Host merges per-core partial y (pure row selection).
"""

import sys
import types

import numpy as np

if "antenv.axon_hooks" not in sys.modules:
    _m = types.ModuleType("antenv.axon_hooks")
    _m.get_axon_ntff_profile_hook = lambda: None
    sys.modules["antenv.axon_hooks"] = _m

import concourse.bass as bass
import concourse.bacc as bacc
import concourse.tile as tile
from concourse import mybir

# ---------------------------------------------------------------------------
N = 2048
C = 2
T = 3
L = 2
E = 65536
W_IN = 256
W_OUT = 64
NCLS = 16
NTGT = 512
BETA = 0.5

NCORES = 8
NS = N // NCORES
P = 128
KC = 16
NEL = 1024
NCH = 12                 # scatter chunks per E-set (T*2*2048 / 1024)
EFREE = T * 2 * 2048     # 12288 free elems per E-set tile

f32 = mybir.dt.float32
f16 = mybir.dt.float16
i16 = mybir.dt.int16
i32 = mybir.dt.int32
AF = mybir.ActivationFunctionType
OP = mybir.AluOpType

# misc pack offsets (columns in the [128, MISC_W] fp32 misc tensor)
_MO_CONV = 0          # [12]
_MO_B1 = 12           # [16]
_MO_B2 = 28           # [64]
_MO_LB = 92           # [16]
_MO_ID = 108          # [128]
_MO_GW1 = 236         # [16]  (partitions 0..63)
_MO_GW2 = 252         # [64]  (partitions 0..15)
_MO_LW = 316          # [16]  (partitions 0..63)
_MO_TIDX = 332        # [1]   (int32 bits: local row of target slot)
MISC_W = 334


# ---------------------------------------------------------------------------
# Host-side packing (indexing / bucketing only; no value arithmetic).
# ---------------------------------------------------------------------------
def _bucket_pack(rank, p_of, free, v_all):
    """Generic per-core scatter packing for one E-set.

    Returns (scat_idx [8,P,NCH*NI] i16, plane0 [8,P,NCH*NI] f32,
             dup [8,max(M-1,1),P,NCH*max(DUPW,1)] f32, NI, M, DUPW)."""
    ch_of = free // NEL
    pos_of = free % NEL
    bucket = (rank * P + p_of) * NCH + ch_of
    cell = bucket * NEL + pos_of

    order = np.argsort(cell, kind="stable")
    cell_s = cell[order]
    v_s = v_all[order]
    ucell, first_idx, counts = np.unique(cell_s, return_index=True,
                                         return_counts=True)
    occ = np.arange(len(cell_s)) - np.repeat(first_idx, counts)
    M = int(counts.max())
    ubucket = ucell // NEL
    upos = ucell % NEL
    # duplicated cells first within each bucket -> dup slots < DUPW
    order2 = np.lexsort((np.arange(len(ucell)), counts == 1, ubucket))
    inv2 = np.empty_like(order2)
    inv2[order2] = np.arange(len(order2))
    ub_sorted = ubucket[order2]
    ub_uniq, ub_fidx, ub_counts = np.unique(ub_sorted, return_index=True,
                                            return_counts=True)
    slot_sorted = np.arange(len(ucell)) - np.repeat(ub_fidx, ub_counts)
    slot = slot_sorted[inv2]
    ndup = np.zeros(len(ub_uniq), np.int64)
    isdup_sorted = (counts[order2] >= 2)
    np.add.at(ndup, np.searchsorted(ub_uniq, ub_sorted), isdup_sorted)
    DUPW = int(ndup.max()) if M > 1 else 0
    max_cnt = int(ub_counts.max())
    NI = max_cnt + (max_cnt & 1)

    scat_idx = np.full((NCORES, P, NCH * NI), -1, np.int16)
    plane0 = np.zeros((NCORES, P, NCH * NI), np.float32)
    dup = np.zeros((NCORES, max(M - 1, 1), P, NCH * max(DUPW, 1)),
                   np.float32)
    uk = ubucket // (P * NCH)
    up = (ubucket // NCH) % P
    uch = ubucket % NCH
    scat_idx[uk, up, uch * NI + slot] = upos.astype(np.int16)
    cell_row = np.searchsorted(ucell, cell_s)
    kk = uk[cell_row]
    pp2 = up[cell_row]
    cc = uch[cell_row]
    ss = slot[cell_row]
    m0 = occ == 0
    plane0[kk[m0], pp2[m0], cc[m0] * NI + ss[m0]] = v_s[m0]
    md = ~m0
    if md.any():
        assert (ss[md] < DUPW).all()
        dup[kk[md], occ[md] - 1, pp2[md], cc[md] * DUPW + ss[md]] = v_s[md]
    return scat_idx, plane0, dup, NI, M, DUPW


def _fold(a):  # [256, F] -> [128, 2*F]  (feat = kf*128 + p)
    fdim = a.shape[1]
    return np.ascontiguousarray(
        a.reshape(2, P, fdim).transpose(1, 0, 2).reshape(P, 2 * fdim))


def _prep_inputs(X, edge_value, conv_w, Ws, gcn_w1, gcn_b1, gcn_w2, gcn_b2,
                 lin_w, lin_b, edge_index, target_x):
    X = np.asarray(X, np.float32)
    edge_value = np.asarray(edge_value, np.float32)
    conv_w = np.asarray(conv_w, np.float32)
    Ws = np.asarray(Ws, np.float32)
    gcn_w1 = np.asarray(gcn_w1, np.float32)
    gcn_b1 = np.asarray(gcn_b1, np.float32)
    gcn_w2 = np.asarray(gcn_w2, np.float32)
    gcn_b2 = np.asarray(gcn_b2, np.float32)
    lin_w = np.asarray(lin_w, np.float32)
    lin_b = np.asarray(lin_b, np.float32)
    ei = np.asarray(edge_index, np.int64)
    tx = np.asarray(target_x, np.int64)

    # xT_perm[:, kc*128 + p] = X.T[:, p*16 + kc]
    kk, pp = np.meshgrid(np.arange(KC), np.arange(P), indexing="ij")
    pos_node = (pp * KC + kk).reshape(-1)
    xT_perm = np.ascontiguousarray(X[pos_node].T)          # [256, 2048]
    ws_cat = np.concatenate([Ws[0], Ws[1]], axis=1)        # [256, 128]

    misc = np.zeros((P, MISC_W), np.float32)
    misc[:, _MO_CONV:_MO_CONV + 12] = conv_w.reshape(1, -1)
    misc[:, _MO_B1:_MO_B1 + 16] = gcn_b1.reshape(1, -1)
    misc[:, _MO_B2:_MO_B2 + 64] = gcn_b2.reshape(1, -1)
    misc[:, _MO_LB:_MO_LB + 16] = lin_b.reshape(1, -1)
    misc[:, _MO_ID:_MO_ID + 128] = np.eye(P, dtype=np.float32)
    misc[:64, _MO_GW1:_MO_GW1 + 16] = gcn_w1
    misc[:16, _MO_GW2:_MO_GW2 + 64] = gcn_w2
    misc[:64, _MO_LW:_MO_LW + 16] = lin_w

    # ---- edge bucketing (both shardings) ----------------------------------
    t_id = np.repeat(np.arange(T, dtype=np.int64), E)
    r_all = ei[:, 0, :].reshape(-1)
    c_all = ei[:, 1, :].reshape(-1)
    v_all = edge_value.reshape(-1)

    # E0 row-shard: lhsT[(t,e,kc)][p, i] = E_t[row=i*16+2k+e, col=p*16+kc]
    rank0 = (r_all % 16) >> 1
    p0 = c_all >> 4
    free0 = t_id * 4096 + (r_all & 1) * 2048 + (c_all % 16) * 128 \
        + (r_all >> 4)
    sidx0, pl0, dup0, NI0, M0, DUPW0 = _bucket_pack(rank0, p0, free0, v_all)

    # E1 col-shard: lhsT[(t,ec,kc)][p, i] = E_t[row=i*16+kc, col=p*16+2k+ec]
    rank1 = (c_all % 16) >> 1
    p1 = c_all >> 4
    free1 = t_id * 4096 + (c_all & 1) * 2048 + (r_all % 16) * 128 \
        + (r_all >> 4)
    sidx1, pl1, dup1, NI1, M1, DUPW1 = _bucket_pack(rank1, p1, free1, v_all)

    # ---- per-core target slots --------------------------------------------
    k_of_t = (tx % 16) >> 1
    row_loc = ((tx >> 4) * 2 + (tx & 1)).astype(np.int32)
    tslots = []   # (positions in y, local rows)
    for k in range(NCORES):
        pos = np.nonzero(k_of_t == k)[0]
        assert len(pos) <= P, f"core {k} owns {len(pos)} targets > {P}"
        rows = np.zeros(P, np.int32)
        rows[:len(pos)] = row_loc[pos]
        tslots.append((pos, rows))

    in_maps = []
    for k in range(NCORES):
        # shard rows ordered (e, p): node = p*16 + 2k + e
        ee, pp2 = np.meshgrid(np.arange(2), np.arange(P), indexing="ij")
        nodes = (pp2 * 16 + 2 * k + ee).reshape(-1)
        xmy = _fold(np.ascontiguousarray(X[nodes].T))      # [128, 512]
        big0 = np.concatenate(
            [_fold(xT_perm), _fold(ws_cat), xmy], axis=1).astype(np.float16)
        mk = misc.copy()
        mk[:, _MO_TIDX] = tslots[k][1].view(np.float32)
        m = {
            "big0": np.ascontiguousarray(big0),
            "misc": mk,
            "sidx0": sidx0[k], "sv0": pl0[k],
            "sdup0": np.ascontiguousarray(
                dup0[k].transpose(1, 0, 2).reshape(P, -1)),
            "sidx1": sidx1[k], "sv1": pl1[k],
            "sdup1": np.ascontiguousarray(
                dup1[k].transpose(1, 0, 2).reshape(P, -1)),
        }
        in_maps.append(m)
    meta = (NI0, M0, DUPW0, NI1, M1, DUPW1)
    return in_maps, meta, tslots


# ---------------------------------------------------------------------------
# Device kernel.
# ---------------------------------------------------------------------------
class _StageStop(Exception):
    pass


def build_kernel(meta, reps=1, stop_after=None):
    NI0, M0, DUPW0, NI1, M1, DUPW1 = meta
    nc = bacc.Bacc("TRN2", target_bir_lowering=False, debug=False,
                   num_devices=NCORES)
    F0 = NCH * NI0
    F1 = NCH * NI1
    XT_OFF = 0
    WS_OFF = 2 * N
    XMY_OFF = 2 * N + 2 * C * W_OUT
    BIG0_W = XMY_OFF + 2 * NS

    big0_d = nc.dram_tensor("big0", [P, BIG0_W], f16, kind="ExternalInput")
    misc_d = nc.dram_tensor("misc", [P, MISC_W], f32, kind="ExternalInput")
    sidx0_d = nc.dram_tensor("sidx0", [P, F0], i16, kind="ExternalInput")
    sv0_d = nc.dram_tensor("sv0", [P, F0], f32, kind="ExternalInput")
    sd0_d = nc.dram_tensor("sdup0", [P, max(M0 - 1, 1) * NCH *
                                     max(DUPW0, 1)], f32,
                           kind="ExternalInput")
    sidx1_d = nc.dram_tensor("sidx1", [P, F1], i16, kind="ExternalInput")
    sv1_d = nc.dram_tensor("sv1", [P, F1], f32, kind="ExternalInput")
    sd1_d = nc.dram_tensor("sdup1", [P, max(M1 - 1, 1) * NCH *
                                     max(DUPW1, 1)], f32,
                           kind="ExternalInput")
    y_d = nc.dram_tensor("y", [NS, NCLS], f32, kind="ExternalOutput")

    ccds = []
    for r in range(reps):
        cc = {
            "rs1i": nc.dram_tensor(f"rs1i{r}", [N, 130], f32),
            "rs1o": nc.dram_tensor(f"rs1o{r}", [NS, 130], f32),
            "ag1i": nc.dram_tensor(f"ag1i{r}", [NS, 17], f32),
            "ag1o": nc.dram_tensor(f"ag1o{r}", [N, 17], f32,
                                   addr_space="Shared"),
            "rs2i": nc.dram_tensor(f"rs2i{r}", [N, 16], f32),
            "rs2o": nc.dram_tensor(f"rs2o{r}", [NS, 16], f32),
            "ag2i": nc.dram_tensor(f"ag2i{r}", [NS, W_OUT], f32),
            "ag2o": nc.dram_tensor(f"ag2o{r}", [N, W_OUT], f32,
                                   addr_space="Shared"),
            "rs3i": nc.dram_tensor(f"rs3i{r}", [N, W_OUT], f32),
            "rs3o": nc.dram_tensor(f"rs3o{r}", [NS, W_OUT], f32),
        }
        ccds.append(cc)
    rg = [list(range(NCORES))]

    # scatter chunk order (see free-layout): E0 e=0 chunks first, then e=1;
    # E1 low-kc chunks first.
    E0_ORDER = [0, 1, 4, 5, 8, 9, 2, 3, 6, 7, 10, 11]
    E1_ORDER = [0, 2, 4, 6, 8, 10, 1, 3, 5, 7, 9, 11]

    with tile.TileContext(nc) as tc:
        import contextlib
        ctx = contextlib.ExitStack()
        with ctx:
            pool = ctx.enter_context(tc.tile_pool(name="main", bufs=1))
            ppool = ctx.enter_context(
                tc.tile_pool(name="pass_psum", bufs=4, space="PSUM"))
            apool = ctx.enter_context(
                tc.tile_pool(name="aux_psum", bufs=3, space="PSUM"))

            # ---------------- input loads ----------------
            misc = pool.tile([P, MISC_W], f32, tag="misc")
            nc.sync.dma_start(misc[:], misc_d[:])
            sv0_sb = pool.tile([P, F0], f32, tag="sv0")
            nc.sync.dma_start(sv0_sb[:], sv0_d[:])
            sidx0_sb = pool.tile([P, F0], i16, tag="sidx0")
            nc.sync.dma_start(sidx0_sb[:], sidx0_d[:])
            sd0_sb = pool.tile([P, sd0_d.shape[1]], f32, tag="sd0")
            nc.sync.dma_start(sd0_sb[:], sd0_d[:])
            sv1_sb = pool.tile([P, F1], f32, tag="sv1")
            nc.sync.dma_start(sv1_sb[:], sv1_d[:])
            sidx1_sb = pool.tile([P, F1], i16, tag="sidx1")
            nc.sync.dma_start(sidx1_sb[:], sidx1_d[:])
            sd1_sb = pool.tile([P, sd1_d.shape[1]], f32, tag="sd1")
            nc.sync.dma_start(sd1_sb[:], sd1_d[:])
            big0 = pool.tile([P, BIG0_W], f16, tag="big0")
            nc.sync.dma_start(big0[:], big0_d[:])

            ident = misc[:, _MO_ID:_MO_ID + 128]
            b1_ap = misc[:, _MO_B1:_MO_B1 + 16]
            b2_ap = misc[:, _MO_B2:_MO_B2 + 64]
            lb_ap = misc[:, _MO_LB:_MO_LB + 16]
            gw1_ap = misc[0:64, _MO_GW1:_MO_GW1 + 16]
            gw2_ap = misc[0:16, _MO_GW2:_MO_GW2 + 64]
            lw_ap = misc[0:64, _MO_LW:_MO_LW + 16]
            tidx_ap = misc[:, _MO_TIDX:_MO_TIDX + 1].bitcast(i32)

            # persistent tiles
            e0sb = pool.tile([P, EFREE], f16, tag="e0sb")
            e1sb = pool.tile([P, EFREE], f16, tag="e1sb")
            rhs_a = pool.tile([P, KC, 130], f16, tag="rhs_a")
            nc.vector.memset(rhs_a[:], 1.0)

            prev_y = None
            stage_state = {}

            def _stage(name, tile_ref):
                stage_state["last"] = tile_ref
                if stop_after == name:
                    raise _StageStop()

            for rep in range(reps):
                try:
                    # ---------- filt = softmax(conv_w) ----------
                    ex = pool.tile([P, L * C * T], f32, tag="ex")
                    nc.scalar.activation(ex[:],
                                         misc[:, _MO_CONV:_MO_CONV + 12],
                                         AF.Exp)
                    sums = pool.tile([P, L * C], f32, tag="sums")
                    nc.vector.tensor_reduce(
                        sums[:], ex[:].rearrange("p (g t) -> p g t", t=T),
                        axis=mybir.AxisListType.X, op=OP.add)
                    rec = pool.tile([P, L * C], f32, tag="rec")
                    nc.vector.reciprocal(rec[:], sums[:])
                    filt = pool.tile([P, L * C * T], f32, tag="filt")
                    for g in range(L * C):
                        nc.vector.tensor_scalar_mul(
                            filt[:, g * T:(g + 1) * T],
                            ex[:, g * T:(g + 1) * T], rec[:, g:g + 1])

                    def fs(l, c, t):
                        q = (l * C + c) * T + t
                        return filt[:, q:q + 1]

                    # ---------- dup-sum (rep 0) + serialization ----------
                    if rep == 0:
                        for (sv, sd, M_, DUPW_, NI_) in (
                                (sv0_sb, sd0_sb, M0, DUPW0, NI0),
                                (sv1_sb, sd1_sb, M1, DUPW1, NI1)):
                            if M_ > 1 and DUPW_ > 0:
                                vv = sv[:].rearrange("p (c s) -> p c s",
                                                     c=NCH)
                                dd = sd[:].rearrange("p (m c s) -> p m c s",
                                                     m=M_ - 1, c=NCH)
                                for m in range(M_ - 1):
                                    nc.vector.tensor_add(
                                        vv[:, :, 0:DUPW_],
                                        vv[:, :, 0:DUPW_], dd[:, m])
                    if prev_y is not None:
                        jz = pool.tile([P, 1], f32, tag="jz")
                        nc.vector.tensor_scalar_mul(jz[:], prev_y, 0.0)
                        nc.vector.tensor_scalar_add(sv0_sb[:, 0:1],
                                                    sv0_sb[:, 0:1], jz[:, :])
                        nc.vector.tensor_scalar_add(sv1_sb[:, 0:1],
                                                    sv1_sb[:, 0:1], jz[:, :])
                        nc.vector.tensor_scalar_add(big0[:, 0:1],
                                                    big0[:, 0:1], jz[:, :])

                    # ---------- quantize to fp16 + scatter ----------
                    vq0 = pool.tile([P, F0], f16, tag="vq0")
                    nc.scalar.activation(vq0[:], sv0_sb[:], AF.Copy)
                    vq1 = pool.tile([P, F1], f16, tag="vq1")
                    nc.scalar.activation(vq1[:], sv1_sb[:], AF.Copy)
                    for ch in E0_ORDER:
                        nc.gpsimd.local_scatter(
                            out_ap=e0sb[:, ch * NEL:(ch + 1) * NEL],
                            data_ap=vq0[:, ch * NI0:(ch + 1) * NI0],
                            idxs_ap=sidx0_sb[:, ch * NI0:(ch + 1) * NI0],
                            channels=P, num_elems=NEL, num_idxs=NI0)
                    for ch in E1_ORDER:
                        nc.gpsimd.local_scatter(
                            out_ap=e1sb[:, ch * NEL:(ch + 1) * NEL],
                            data_ap=vq1[:, ch * NI1:(ch + 1) * NI1],
                            idxs_ap=sidx1_sb[:, ch * NI1:(ch + 1) * NI1],
                            channels=P, num_elems=NEL, num_idxs=NI1)
                    _stage("ebuild", e1sb[:, 0:1])

                    def e0chunk(t, e, kc):
                        o = t * 4096 + e * 2048 + kc * 128
                        return e0sb[:, o:o + 128]

                    def e1chunk(t, ec, kc):
                        o = t * 4096 + ec * 2048 + kc * 128
                        return e1sb[:, o:o + 128]

                    # ---------- rhs_a = [X_|1 | X_|1] fp16 ----------
                    for kc in range(KC):
                        ps = apool.tile([P, C * W_OUT], f32, space="PSUM",
                                        tag="aux")
                        for a in range(2):
                            nc.tensor.matmul(
                                ps[:],
                                big0[:, XT_OFF + a * N + kc * P:
                                     XT_OFF + a * N + (kc + 1) * P],
                                big0[:, WS_OFF + a * C * W_OUT:
                                     WS_OFF + (a + 1) * C * W_OUT],
                                start=(a == 0), stop=(a == 1))
                        nc.scalar.activation(
                            rhs_a[:, kc, :].rearrange(
                                "p (b q) -> p b q", q=65)[:, :, 0:64],
                            ps[:].rearrange("p (b q) -> p b q", q=64),
                            AF.Copy)

                    # X_sh for the Hc tail: [128, 2, 128] fp32
                    xsh = pool.tile([P, 2, C * W_OUT], f32, tag="xsh")
                    for e in range(2):
                        ps = apool.tile([P, C * W_OUT], f32, space="PSUM",
                                        tag="aux")
                        for a in range(2):
                            nc.tensor.matmul(
                                ps[:],
                                big0[:, XMY_OFF + a * 2 * P + e * P:
                                     XMY_OFF + a * 2 * P + (e + 1) * P],
                                big0[:, WS_OFF + a * C * W_OUT:
                                     WS_OFF + (a + 1) * C * W_OUT],
                                start=(a == 0), stop=(a == 1))
                        nc.vector.tensor_copy(xsh[:, e, :], ps[:])

                    # ---------- xs[t] = per-type scaled rhs_a ----------
                    xs = [pool.tile([P, KC, 130], f16, tag=f"xs{t}",
                                    name=f"xs{t}")
                          for t in range(T)]
                    for t in range(T):
                        for c in range(C):
                            nc.vector.tensor_scalar_mul(
                                xs[t][:, :, c * 65:(c + 1) * 65],
                                rhs_a[:, :, c * 65:(c + 1) * 65],
                                fs(0, c, t))

                    # ================ PASS A0 ================
                    s0 = pool.tile([P, 2, 130], f16, tag="s0")
                    for e in range(2):
                        ps = ppool.tile([P, 130], f32, space="PSUM",
                                        tag="ep")
                        first = True
                        for t in range(T):
                            for kc in range(KC):
                                nc.tensor.matmul(
                                    ps[:], e0chunk(t, e, kc),
                                    xs[t][:, kc, :], start=first,
                                    stop=(t == T - 1 and kc == KC - 1))
                                first = False
                        nc.scalar.activation(s0[:, e, :], ps[:], AF.Copy)

                    # ss[t] = per-type scaled s0
                    ss = [pool.tile([P, 2, 130], f16, tag=f"ss{t}",
                                    name=f"ss{t}")
                          for t in range(T)]
                    for t in range(T):
                        for c in range(C):
                            nc.vector.tensor_scalar_mul(
                                ss[t][:, :, c * 65:(c + 1) * 65],
                                s0[:, :, c * 65:(c + 1) * 65], fs(1, c, t))

                    # ================ PASS A1 ================
                    stg1 = pool.tile([P, KC, 130], f32, tag="stg1")
                    for kc in range(KC):
                        ps = ppool.tile([P, 130], f32, space="PSUM",
                                        tag="ep")
                        first = True
                        for t in range(T):
                            for ec in range(2):
                                nc.tensor.matmul(
                                    ps[:], e1chunk(t, ec, kc),
                                    ss[t][:, ec, :], start=first,
                                    stop=(t == T - 1 and ec == 1))
                                first = False
                        nc.scalar.activation(stg1[:, kc, :], ps[:], AF.Copy)
                    _stage("passA", stg1[:, 0, 0:1])

                    cc = ccds[rep]

                    def rs_round(name, stage_sb, d, cin, cout):
                        cv = cin[:].rearrange("(g p e) d -> g p e d", p=P,
                                              e=2)
                        for g in range(NCORES):
                            nc.sync.dma_start(cv[g],
                                              stage_sb[:, 2 * g:2 * g + 2,
                                                       :])
                        nc.gpsimd.collective_compute(
                            "ReduceScatter", OP.add, replica_groups=rg,
                            ins=[cin[:]], outs=[cout[:]])
                        rsh = pool.tile([P, 2, d], f32, tag=f"rsh_{name}")
                        nc.sync.dma_start(
                            rsh[:],
                            cout[:].rearrange("(p e) d -> p e d", e=2))
                        return rsh

                    rsh1 = rs_round("1", stg1, 130, cc["rs1i"], cc["rs1o"])
                    _stage("rs1", rsh1[:, 0, 0:1])

                    # ---------- round-1 tail: deg/dinv, Hc, W1 ----------
                    dg = pool.tile([P, 2], f32, tag="deg")
                    nc.vector.tensor_add(dg[:], rsh1[:, :, 64],
                                         rsh1[:, :, 129])
                    nc.vector.tensor_scalar_add(dg[:], dg[:], 1.0)
                    sq = pool.tile([P, 2], f32, tag="sq")
                    nc.scalar.activation(sq[:], dg[:], AF.Sqrt)
                    dinv = pool.tile([P, 2], f32, tag="dinv")
                    nc.vector.reciprocal(dinv[:], sq[:])

                    hcT = pool.tile([W_OUT, 2 * P], f32, tag="hcT")
                    for e in range(2):
                        t1 = pool.tile([P, 2, W_OUT], f32, tag="hct1",
                                       bufs=2)
                        nc.vector.tensor_add(
                            t1[:],
                            xsh[:, e, :].rearrange("p (b q) -> p b q",
                                                   q=64),
                            rsh1[:, e, :].rearrange(
                                "p (b q) -> p b q", q=65)[:, :, 0:64])
                        r1 = pool.tile([P, 2, W_OUT], f32, tag="hcr1",
                                       bufs=2)
                        nc.scalar.activation(r1[:], t1[:], AF.Relu,
                                             scale=BETA)
                        hc_e = pool.tile([P, W_OUT], f32, tag="hc_e",
                                         bufs=2)
                        nc.vector.tensor_add(hc_e[:], r1[:, 0, :],
                                             r1[:, 1, :])
                        nc.vector.tensor_scalar_mul(hc_e[:], hc_e[:], 0.5)
                        tp = apool.tile([P, P], f32, space="PSUM",
                                        tag="aux")
                        nc.tensor.transpose(tp[:W_OUT, :], hc_e[:], ident)
                        nc.vector.tensor_copy(hcT[:, e * P:(e + 1) * P],
                                              tp[:W_OUT, :])
                    w1d = pool.tile([P, 2, 17], f32, tag="w1d")
                    for e in range(2):
                        psz = apool.tile([P, 16], f32, space="PSUM",
                                         tag="aux")
                        nc.tensor.matmul(psz[:], hcT[:, e * P:(e + 1) * P],
                                         gw1_ap, start=True, stop=True)
                        nc.vector.tensor_scalar_mul(w1d[:, e, 0:16], psz[:],
                                                    dinv[:, e:e + 1])
                    nc.vector.tensor_copy(w1d[:, :, 16], dinv[:, :])

                    def ag_round(name, shard_sb, d, cin, cout):
                        nc.sync.dma_start(
                            cin[:].rearrange("(p e) d -> p e d", e=2),
                            shard_sb[:])
                        nc.gpsimd.collective_compute(
                            "AllGather", OP.bypass, replica_groups=rg,
                            ins=[cin[:]], outs=[cout[:]])
                        rhs = pool.tile([P, KC, d], f32, tag=f"rhs_{name}")
                        cov = cout[:].rearrange("(g p e) d -> g p e d",
                                                p=P, e=2)
                        for g in range(NCORES):
                            nc.sync.dma_start(rhs[:, 2 * g:2 * g + 2, :],
                                              cov[g])
                        return rhs

                    rhs_c = ag_round("c", w1d, 17, cc["ag1i"], cc["ag1o"])
                    _stage("ag1", rhs_c[:, 0, 0:1])

                    # ---------- ws1[t]; PASS C0 ----------
                    ws1 = [pool.tile([P, KC, 2 * 16], f16, tag=f"ws1{t}",
                                     name=f"ws1{t}")
                           for t in range(T)]
                    for t in range(T):
                        for c in range(C):
                            nc.vector.tensor_scalar_mul(
                                ws1[t][:, :, c * 16:(c + 1) * 16],
                                rhs_c[:, :, 0:16], fs(0, c, t))
                    s1 = pool.tile([P, 2, 2 * 16], f16, tag="s1")
                    for e in range(2):
                        ps = ppool.tile([P, 2 * 16], f32, space="PSUM",
                                        tag="ep")
                        first = True
                        for t in range(T):
                            for kc in range(KC):
                                nc.tensor.matmul(
                                    ps[:], e0chunk(t, e, kc),
                                    ws1[t][:, kc, :], start=first,
                                    stop=(t == T - 1 and kc == KC - 1))
                                first = False
                        nc.scalar.activation(s1[:, e, :], ps[:], AF.Copy)

                    # rhsD[t] = fs(1,0,t)*s1_c0 + fs(1,1,t)*s1_c1
                    rhsD = [pool.tile([P, 2, 16], f16, tag=f"rhsD{t}",
                                      name=f"rhsD{t}")
                            for t in range(T)]
                    for t in range(T):
                        tmp = pool.tile([P, 2, 16], f16, tag="tmpD", bufs=2)
                        nc.vector.tensor_scalar_mul(
                            tmp[:], s1[:, :, 0:16], fs(1, 0, t))
                        nc.vector.scalar_tensor_tensor(
                            out=rhsD[t][:], in0=s1[:, :, 16:32],
                            scalar=fs(1, 1, t), in1=tmp[:],
                            op0=OP.mult, op1=OP.add)

                    # ================ PASS D1 ================
                    stgD = pool.tile([P, KC, 16], f32, tag="stgD")
                    for kc in range(KC):
                        ps = ppool.tile([P, 16], f32, space="PSUM",
                                        tag="ep")
                        first = True
                        for t in range(T):
                            for ec in range(2):
                                nc.tensor.matmul(
                                    ps[:], e1chunk(t, ec, kc),
                                    rhsD[t][:, ec, :], start=first,
                                    stop=(t == T - 1 and ec == 1))
                                first = False
                        nc.scalar.activation(stgD[:, kc, :], ps[:], AF.Copy)
                    _stage("passD", stgD[:, 0, 0:1])

                    rsh2 = rs_round("2", stgD, 16, cc["rs2i"], cc["rs2o"])
                    _stage("rs2", rsh2[:, 0, 0:1])

                    # ---------- round-2 tail: h, W2 ----------
                    hT = pool.tile([16, 2 * P], f32, tag="hT")
                    w2sh = pool.tile([P, 2, W_OUT], f32, tag="w2sh")
                    for e in range(2):
                        aw = pool.tile([P, 16], f32, tag="aw1", bufs=2)
                        nc.vector.tensor_add(aw[:], rsh2[:, e, :],
                                             w1d[:, e, 0:16])
                        nc.vector.scalar_tensor_tensor(
                            out=aw[:], in0=aw[:], scalar=dinv[:, e:e + 1],
                            in1=b1_ap, op0=OP.mult, op1=OP.add)
                        h_e = pool.tile([P, 16], f32, tag="h_e", bufs=2)
                        nc.vector.tensor_scalar_max(h_e[:], aw[:], 0.0)
                        tp = apool.tile([P, P], f32, space="PSUM",
                                        tag="aux")
                        nc.tensor.transpose(tp[:16, :], h_e[:], ident)
                        nc.vector.tensor_copy(hT[:, e * P:(e + 1) * P],
                                              tp[:16, :])
                    for e in range(2):
                        psz = apool.tile([P, W_OUT], f32, space="PSUM",
                                         tag="aux")
                        nc.tensor.matmul(psz[:], hT[:, e * P:(e + 1) * P],
                                         gw2_ap, start=True, stop=True)
                        nc.vector.tensor_scalar_mul(w2sh[:, e, :], psz[:],
                                                    dinv[:, e:e + 1])

                    rhs_e = ag_round("e", w2sh, W_OUT, cc["ag2i"],
                                     cc["ag2o"])
                    _stage("ag2", rhs_e[:, 0, 0:1])

                    # ---------- ws2[t]; PASS E0 ----------
                    re16 = pool.tile([P, KC, W_OUT], f16, tag="re16")
                    nc.scalar.activation(re16[:], rhs_e[:], AF.Copy)
                    ws2 = [pool.tile([P, KC, 2 * W_OUT], f16,
                                     tag=f"ws2{t}", name=f"ws2{t}")
                           for t in range(T)]
                    for t in range(T):
                        for c in range(C):
                            eng = nc.vector if (t * C + c) % 2 == 0 \
                                else nc.scalar
                            if eng is nc.vector:
                                nc.vector.tensor_scalar_mul(
                                    ws2[t][:, :, c * 64:(c + 1) * 64],
                                    re16[:], fs(0, c, t))
                            else:
                                nc.scalar.activation(
                                    ws2[t][:, :, c * 64:(c + 1) * 64],
                                    re16[:], AF.Copy, scale=fs(0, c, t))
                    s2 = pool.tile([P, 2, 2 * W_OUT], f16, tag="s2")
                    for e in range(2):
                        ps = ppool.tile([P, 2 * W_OUT], f32, space="PSUM",
                                        tag="ep")
                        first = True
                        for t in range(T):
                            for kc in range(KC):
                                nc.tensor.matmul(
                                    ps[:], e0chunk(t, e, kc),
                                    ws2[t][:, kc, :], start=first,
                                    stop=(t == T - 1 and kc == KC - 1))
                                first = False
                        nc.scalar.activation(s2[:, e, :], ps[:], AF.Copy)

                    rhsF = [pool.tile([P, 2, W_OUT], f16, tag=f"rhsF{t}",
                                      name=f"rhsF{t}")
                            for t in range(T)]
                    for t in range(T):
                        tmp = pool.tile([P, 2, W_OUT], f16, tag="tmpF",
                                        bufs=2)
                        nc.vector.tensor_scalar_mul(
                            tmp[:], s2[:, :, 0:W_OUT], fs(1, 0, t))
                        nc.vector.scalar_tensor_tensor(
                            out=rhsF[t][:], in0=s2[:, :, W_OUT:2 * W_OUT],
                            scalar=fs(1, 1, t), in1=tmp[:],
                            op0=OP.mult, op1=OP.add)

                    # ================ PASS F1 ================
                    stgF = pool.tile([P, KC, W_OUT], f32, tag="stgF")
                    for kc in range(KC):
                        ps = ppool.tile([P, W_OUT], f32, space="PSUM",
                                        tag="ep")
                        first = True
                        for t in range(T):
                            for ec in range(2):
                                nc.tensor.matmul(
                                    ps[:], e1chunk(t, ec, kc),
                                    rhsF[t][:, ec, :], start=first,
                                    stop=(t == T - 1 and ec == 1))
                                first = False
                        nc.scalar.activation(stgF[:, kc, :], ps[:], AF.Copy)
                    _stage("passF", stgF[:, 0, 0:1])

                    rsh3 = rs_round("3", stgF, W_OUT, cc["rs3i"],
                                    cc["rs3o"])
                    _stage("rs3", rsh3[:, 0, 0:1])

                    # ---------- round-3 tail: log_softmax + head ----------
                    hls = pool.tile([P, 2, W_OUT], f32, tag="hls")
                    for e in range(2):
                        aw = pool.tile([P, W_OUT], f32, tag="aw2", bufs=2)
                        nc.vector.tensor_add(aw[:], rsh3[:, e, :],
                                             w2sh[:, e, :])
                        nc.vector.scalar_tensor_tensor(
                            out=aw[:], in0=aw[:], scalar=dinv[:, e:e + 1],
                            in1=b2_ap, op0=OP.mult, op1=OP.add)
                        mx = pool.tile([P, 1], f32, tag="mx", bufs=2)
                        nc.vector.tensor_reduce(mx[:], aw[:],
                                                axis=mybir.AxisListType.X,
                                                op=OP.max)
                        nmx = pool.tile([P, 1], f32, tag="nmx", bufs=2)
                        nc.vector.tensor_scalar_mul(nmx[:], mx[:], -1.0)
                        ee = pool.tile([P, W_OUT], f32, tag="ee", bufs=2)
                        nc.scalar.activation(ee[:], aw[:], AF.Exp,
                                             bias=nmx[:, :])
                        ssum = pool.tile([P, 1], f32, tag="ssum", bufs=2)
                        nc.vector.tensor_reduce(ssum[:], ee[:],
                                                axis=mybir.AxisListType.X,
                                                op=OP.add)
                        lns = pool.tile([P, 1], f32, tag="lns", bufs=2)
                        nc.scalar.activation(lns[:], ssum[:], AF.Ln)
                        tot = pool.tile([P, 1], f32, tag="tot", bufs=2)
                        nc.vector.tensor_add(tot[:], mx[:], lns[:])
                        nc.vector.tensor_scalar(out=hls[:, e, :],
                                                in0=aw[:],
                                                scalar1=tot[:, :],
                                                scalar2=None,
                                                op0=OP.subtract)

                    # head on ALL 256 local rows; host selects target rows
                    y_sb = pool.tile([P, 2, NCLS], f32, tag="y_sb")
                    for e in range(2):
                        tp = apool.tile([P, P], f32, space="PSUM",
                                        tag="aux")
                        nc.tensor.transpose(tp[:W_OUT, :], hls[:, e, :],
                                            ident)
                        gT = pool.tile([W_OUT, P], f32, tag="gT", bufs=2)
                        nc.vector.tensor_copy(gT[:], tp[:W_OUT, :])
                        psy = apool.tile([P, NCLS], f32, space="PSUM",
                                         tag="aux")
                        nc.tensor.matmul(psy[:], gT[:], lw_ap, start=True,
                                         stop=True)
                        nc.vector.tensor_add(y_sb[:, e, :], psy[:], lb_ap)
                    nc.sync.dma_start(
                        y_d[:].rearrange("(p e) d -> p e d", e=2),
                        y_sb[:])
                except _StageStop:
                    lt = stage_state["last"]
                    y_sb = pool.tile([P, 2, NCLS], f32, tag="ydummy")
                    nc.vector.memset(y_sb[:], 0.0)
                    nc.vector.tensor_scalar_mul(y_sb[:, 0, 0:1], lt, 0.0)
                    nc.sync.dma_start(
                        y_d[:].rearrange("(p e) d -> p e d", e=2),
                        y_sb[:])
                prev_y = y_sb[:, 0, 0:1]

    nc.compile()
    return nc


# ---------------------------------------------------------------------------
# Execution via PJRT (axon).
# ---------------------------------------------------------------------------
class _Runner:
    def __init__(self, nc, n_cores):
        import jax
        from jax.sharding import Mesh, PartitionSpec
        from jax.experimental.shard_map import shard_map
        from concourse.bass2jax import (
            _bass_exec_p, install_neuronx_cc_hook, partition_id_tensor)

        install_neuronx_cc_hook()
        self.jax = jax
        self._nc = nc
        self.n_cores = n_cores
        partition_name = (
            nc.partition_id_tensor.name if nc.partition_id_tensor else None)
        in_names, out_names, out_avals, zero_outs = [], [], [], []
        for alloc in nc.m.functions[0].allocations:
            if not isinstance(alloc, mybir.MemoryLocationSet):
                continue
            name = alloc.memorylocations[0].name
            if alloc.kind == "ExternalInput":
                if name != partition_name:
                    in_names.append(name)
            elif alloc.kind == "ExternalOutput":
                shape = tuple(alloc.tensor_shape)
                dtype = mybir.dt.np(alloc.dtype)
                out_names.append(name)
                out_avals.append(jax.core.ShapedArray(shape, dtype))
                zero_outs.append(np.zeros(shape, dtype))
        self.n_params = len(in_names)
        self.out_names = out_names
        self.out_avals = out_avals
        self.zero_outs = zero_outs
        n_outs = len(out_avals)
        in_names = in_names + out_names
        if partition_name is not None:
            in_names.append(partition_name)
        self.in_names = in_names

        def _body(*args):
            operands = list(args)
            if partition_name is not None:
                operands.append(partition_id_tensor())
            outs = _bass_exec_p.bind(
                *operands, out_avals=tuple(out_avals),
                in_names=tuple(in_names), out_names=tuple(out_names),
                lowering_input_output_aliases=(),
                sim_require_finite=True, sim_require_nnan=True, nc=nc)
            return tuple(outs)

        devices = jax.devices()[:n_cores]
        mesh = Mesh(np.asarray(devices), ("core",))
        in_specs = (PartitionSpec("core"),) * (self.n_params + n_outs)
        out_specs = (PartitionSpec("core"),) * n_outs
        self._fn = jax.jit(
            shard_map(_body, mesh=mesh, in_specs=in_specs,
                      out_specs=out_specs, check_rep=False),
            donate_argnums=tuple(range(self.n_params,
                                       self.n_params + n_outs)),
            keep_unused=True)

    def concat_inputs(self, in_maps):
        return [
            np.concatenate([np.asarray(m[name]) for m in in_maps], axis=0)
            for name in self.in_names[: self.n_params]
        ]

    def zeros(self):
        return [
            np.zeros((self.n_cores * z.shape[0], *z.shape[1:]), z.dtype)
            for z in self.zero_outs
        ]

    def run(self, in_maps):
        outs = self._fn(*self.concat_inputs(in_maps), *self.zeros())
        return [
            {
                name: np.asarray(outs[i]).reshape(
                    self.n_cores, *self.out_avals[i].shape)[c]
                for i, name in enumerate(self.out_names)
            }
            for c in range(self.n_cores)
        ]


_CACHE = {}


def _get_runner(meta, reps=1, stop_after=None):
    key = (meta, reps, stop_after)
    if key not in _CACHE:
        nc = build_kernel(meta, reps=reps, stop_after=stop_after)
        _CACHE[key] = _Runner(nc, NCORES)
    return _CACHE[key]


def kernel(**inputs) -> np.ndarray:
    in_maps, meta, tslots = _prep_inputs(**inputs)
    runner = _get_runner(meta)
    results = runner.run(in_maps)
    y = np.zeros((NTGT, NCLS), np.float32)
    for k in range(NCORES):
        pos, rows = tslots[k]
        y[pos] = results[k]["y"][rows[: len(pos)]]
    return y
